# revision 1
# baseline (speedup 1.0000x reference)
"""DeepSeekMoE (B=2,S=2048,H=1024,I=2816, 7 routed experts top-2 + 1 shared) on 8 trn2 NeuronCores.

Strategy: expert-parallel sparse dispatch.
  - Host computes the router (fp32, 0.01% of FLOPs) and dispatches tokens:
    core c (c<7) owns routed expert c; the largest expert's token list is split
    with core 7 to balance load. Every core also computes the shared expert for
    the 512 output tokens it will own after the ReduceScatters.
  - DMA stalls almost completely while a collective runs, so the schedule keeps
    every post-collective dependency resident:
      1. shared gate/up (weight streaming while HBM is free)
      2. routed gate/up; the shared-expert *down* matmuls are interleaved here
         (streamed sdw chunks, accumulated into SBUF f32) so nothing of the
         shared expert remains after the collectives start
      3. routed down over token half A (tokens < 2048) -> scale -> scatter
         into partial_a [2048,1024] bf16 -> ReduceScatter A
      4. routed down half B (all-resident, overlaps RS_A) -> scatter ->
         ReduceScatter B
      5. out = rs_out + shared_y (A-half adds overlap RS_B)
  - Host concatenates/reorders the 8 [512,1024] output shards.
"""

import math
import os
import sys
import types

import numpy as np
import ml_dtypes

for _p in ('/opt/trn_rl_repo', '/root/.axon_site/_ro/trn_rl_repo'):
    if os.path.isdir(_p) and _p not in sys.path:
        sys.path.append(_p)


def _install_profile_glue():
    """Optional: register the NTFF profile hook so trace=True/BASS_TRACE works
    under axon (the image's antenv lacks axon_hooks). Harmless if unavailable."""
    try:
        import antenv
        if 'antenv.axon_hooks' in sys.modules:
            return
        mod = types.ModuleType('antenv.axon_hooks')
        holder = [None]
        mod.set_axon_ntff_profile_hook = lambda h: holder.__setitem__(0, h)
        mod.get_axon_ntff_profile_hook = lambda: holder[0]
        sys.modules['antenv.axon_hooks'] = mod
        antenv.axon_hooks = mod
        so = '/opt/axon/libaxon_pjrt.so'
        if os.path.exists(so):
            from trn_agent_boot.trn_boot import _ntff_profile_via_ctypes
            hook = _ntff_profile_via_ctypes(so)
            if hook is not None:
                mod.set_axon_ntff_profile_hook(hook)
    except Exception:
        pass


_install_profile_glue()

import concourse.bass as bass
import concourse.mybir as mybir
from concourse.bass_utils import run_bass_kernel_spmd
from concourse.tile import TileContext

B, S, H, I = 2, 2048, 1024, 2816
E_ROUTED = 7
TOP_K = 2
T = B * S                  # 4096 tokens
HALF = T // 2              # token-id split point for the two ReduceScatters
NCORES = 8
SH = T // NCORES           # 512 shared-slice tokens per core
SHH = SH // 2              # 256: per-core output rows from each RS
KH = H // 128              # 8 contraction chunks over H
KI = I // 128              # 22 contraction chunks over I
NH = H // 512              # 2 N-chunks for the down matmul

F32 = mybir.dt.float32
BF16 = mybir.dt.bfloat16
I32 = mybir.dt.int32

PAD_IDX = 1 << 20          # scatter index for pad slots; dropped by bounds_check

LAST_RESULT = None         # BassKernelResults of the most recent run (for tests)

_PROG_CACHE = {}


def _split_sync_waits(nc, max_waits=1):
    """This container's walrus rejects >1 sync wait per instruction; spill
    extra waits onto same-engine NoOps placed just before the instruction."""
    for f in nc.m.functions:
        for bb in f.blocks:
            new_list = []
            changed = False
            for inst in bb.instructions:
                si = inst.sync_info
                if si is not None and si.on_wait is not None and len(si.on_wait) > max_waits:
                    waits = list(si.on_wait)
                    while len(waits) > max_waits:
                        chunk, waits = waits[:max_waits], waits[max_waits:]
                        nop = mybir.InstNoOp(
                            name=nc.get_next_instruction_name(),
                            engine=inst.engine, bass_nofuse=True,
                            sync_info=mybir.SyncInfo(on_wait=chunk, on_update=[]),
                        )
                        new_list.append(nop)
                    inst.sync_info = mybir.SyncInfo(
                        on_wait=waits, on_update=list(si.on_update or []))
                    changed = True
                new_list.append(inst)
            if changed:
                bb.instructions[:] = new_list


def _col_tiles(total, width=512):
    out = []
    c = 0
    while c < total:
        out.append((c, min(width, total - c)))
        c += width
    return out


def _build_program(caps, level=0):
    """Build the SPMD bass program for routed capacities (capA, capB).
    level > 0 progressively frees SBUF (less collective-overlap buffering)
    so unusually skewed routings still compile."""
    capA, capB = caps
    C = capA + capB
    CT = SH + C            # xt columns: shared tokens first, then routed tokens
    NM = C // 128
    NMA = capA // 128

    nc = bass.Bass()
    xt = nc.declare_dram_parameter('xt', [H, CT], BF16, isOutput=False)
    idx = nc.declare_dram_parameter('idx', [C], I32, isOutput=False)
    wv = nc.declare_dram_parameter('wv', [C], F32, isOutput=False)
    # gate/up weights arrive chunk-shuffled: [KI, 128, KH, 128] so each
    # per-I-chunk stream DMA reads 2KB-contiguous per partition.
    gw = nc.declare_dram_parameter('gw', [KI, 128, KH, 128], BF16, isOutput=False)
    uw = nc.declare_dram_parameter('uw', [KI, 128, KH, 128], BF16, isOutput=False)
    dw = nc.declare_dram_parameter('dw', [I, H], BF16, isOutput=False)
    sgw = nc.declare_dram_parameter('sgw', [KI, 128, KH, 128], BF16, isOutput=False)
    suw = nc.declare_dram_parameter('suw', [KI, 128, KH, 128], BF16, isOutput=False)
    sdw = nc.declare_dram_parameter('sdw', [I, H], BF16, isOutput=False)
    out = nc.declare_dram_parameter('out', [SH, H], F32, isOutput=True)

    part = [nc.dram_tensor('part_a', [HALF, H], BF16),
            nc.dram_tensor('part_b', [HALF, H], BF16)]
    rs = [nc.dram_tensor('rs_a', [SHH, H], BF16),
          nc.dram_tensor('rs_b', [SHH, H], BF16)]

    with TileContext(nc) as tc:
        with (
            tc.tile_pool(name='big', bufs=1) as bigp,
            tc.tile_pool(name='wstream', bufs=2 if level < 3 else 1) as wsp,
            tc.tile_pool(name='work', bufs=2) as wkp,
            tc.tile_pool(name='rtp', bufs=2) as rtp,
            tc.tile_pool(name='ps', bufs=8, space='PSUM') as psp,
        ):
            XTs = bigp.tile([128, KH, SH], BF16, tag='XTs')
            XTr = bigp.tile([128, KH, C], BF16, tag='XTr')
            xt_r = xt.rearrange('(k p) c -> p k c', p=128)
            HTs = bigp.tile([128, KI, SH], BF16, tag='HTs')
            HTr = bigp.tile([128, KI, C], BF16, tag='HTr')
            DW = bigp.tile([128, KI, H], BF16, tag='DW')
            IT = bigp.tile([128, NM], I32, tag='IT')
            WT = bigp.tile([128, NM], F32, tag='WT')
            YS = bigp.tile([128, SH // 128, H], F32, tag='YS')
            KDEF = [13, 18, 22, 22][level]   # shared-down chunks deferred to RS_B window
            ZT = bigp.tile([128, H], BF16, tag='DCH2', name='ZT')
            DCH2 = (bigp.tile([128, KI - KDEF, H], BF16, tag='DCH2', name='DCH2')
                    if KDEF < KI else None)

            nc.vector.memset(YS[:, :, :], 0.0)
            nc.vector.memset(ZT[:, :], 0.0)

            def gu_tile(gch, uch, XTt, HTt, i, t0, tn):
                gps = psp.tile([128, 512], F32, tag='ps', name=f'g{i}_{t0}_{tn}')
                ups = psp.tile([128, 512], F32, tag='ps', name=f'u{i}_{t0}_{tn}')
                for k in range(KH):
                    nc.tensor.matmul(
                        gps[:, :tn], lhsT=gch[:, k, :], rhs=XTt[:, k, t0:t0 + tn],
                        start=(k == 0), stop=(k == KH - 1))
                for k in range(KH):
                    nc.tensor.matmul(
                        ups[:, :tn], lhsT=uch[:, k, :], rhs=XTt[:, k, t0:t0 + tn],
                        start=(k == 0), stop=(k == KH - 1))
                at = rtp.tile([128, 512], F32, tag='rt', name=f'at{i}_{t0}_{tn}')
                nc.scalar.activation(
                    out=at[:, :tn], in_=gps[:, :tn],
                    func=mybir.ActivationFunctionType.Silu)
                nc.vector.tensor_tensor(
                    out=HTt[:, i, t0:t0 + tn],
                    in0=at[:, :tn], in1=ups[:, :tn], op=mybir.AluOpType.mult)

            def shared_down_k(k, rhs_tile):
                for m in range(SH // 128):
                    for n in range(NH):
                        ptmp = psp.tile([128, 512], F32, tag='ps', name=f'pt{k}_{m}_{n}')
                        nc.tensor.matmul(
                            ptmp[:, :],
                            lhsT=HTs[:, k, m * 128:(m + 1) * 128],
                            rhs=rhs_tile[:, n * 512:(n + 1) * 512],
                            start=True, stop=True)
                        nc.vector.tensor_tensor(
                            out=YS[:, m, n * 512:(n + 1) * 512],
                            in0=YS[:, m, n * 512:(n + 1) * 512],
                            in1=ptmp[:, :], op=mybir.AluOpType.add)

            # ---- 1. merged gate/up: shared + routed per i-chunk, with the
            #         shared-expert down interleaved (k = i for i < KDEF)
            for i in range(KI):
                sgch = wsp.tile([128, KH, 128], BF16, tag='gch', name=f'sg{i}')
                nc.sync.dma_start(out=sgch[:, :, :], in_=sgw[i, :, :, :])
                such = wsp.tile([128, KH, 128], BF16, tag='uch', name=f'su{i}')
                nc.sync.dma_start(out=such[:, :, :], in_=suw[i, :, :, :])
                if i == 0:
                    # shared x arrives first so PE starts on the shared tile
                    for k in range(KH):
                        nc.sync.dma_start(out=XTs[:, k, :], in_=xt_r[:, k, 0:SH])
                gch = wsp.tile([128, KH, 128], BF16, tag='gch', name=f'rg{i}')
                nc.sync.dma_start(out=gch[:, :, :], in_=gw[i, :, :, :])
                uch = wsp.tile([128, KH, 128], BF16, tag='uch', name=f'ru{i}')
                nc.sync.dma_start(out=uch[:, :, :], in_=uw[i, :, :, :])
                if i == 0:
                    for k in range(KH):
                        nc.sync.dma_start(out=XTr[:, k, :], in_=xt_r[:, k, SH:SH + C])
                gu_tile(sgch, such, XTs, HTs, i, 0, SH)
                for (t0, tn) in _col_tiles(C):
                    gu_tile(gch, uch, XTr, HTr, i, t0, tn)
                if i < KDEF:
                    dch = wsp.tile([128, H], BF16, tag='dch', name=f'dc{i}')
                    nc.sync.dma_start(out=dch[:, :], in_=sdw[i * 128:(i + 1) * 128, :])
                    shared_down_k(i, dch)
                if i == 6:
                    nc.gpsimd.dma_start(out=IT[:, :], in_=idx.rearrange('(m p) -> p m', p=128))
                    nc.gpsimd.dma_start(out=WT[:, :], in_=wv.rearrange('(m p) -> p m', p=128))
                    for h in range(2):
                        for r in range(HALF // 128):
                            nc.gpsimd.dma_start(
                                out=part[h][r * 128:(r + 1) * 128, :], in_=ZT[:, :])
                if i == KDEF and DCH2 is not None:
                    for k in range(KDEF, KI):
                        nc.sync.dma_start(
                            out=DCH2[:, k - KDEF, :], in_=sdw[k * 128:(k + 1) * 128, :])
                if i == KI - 3:
                    for k in range(KI):
                        nc.sync.dma_start(
                            out=DW[:, k, :], in_=dw[k * 128:(k + 1) * 128, :])

            # ---- 2./3. routed down per half + scatter + ReduceScatter
            rt_a = []
            for m in range(NM):
                h = 0 if m < NMA else 1
                ysb = wkp.tile([128, H], BF16, tag='ysb')
                for n in range(NH):
                    yps = psp.tile([128, 512], F32, tag='ps', name=f'y{m}_{n}')
                    for k in range(KI):
                        nc.tensor.matmul(
                            yps[:, :],
                            lhsT=HTr[:, k, m * 128:(m + 1) * 128],
                            rhs=DW[:, k, n * 512:(n + 1) * 512],
                            start=(k == 0), stop=(k == KI - 1))
                    nc.vector.tensor_scalar_mul(
                        ysb[:, n * 512:(n + 1) * 512], yps[:, :], WT[:, m:m + 1])
                nc.gpsimd.indirect_dma_start(
                    out=part[h][:, :],
                    out_offset=bass.IndirectOffsetOnAxis(ap=IT[:, m:m + 1], axis=0),
                    in_=ysb[:, :], in_offset=None,
                    bounds_check=HALF - 1, oob_is_err=False)
                if m == NMA - 1 or m == NM - 1:
                    nc.gpsimd.collective_compute(
                        'ReduceScatter', mybir.AluOpType.add,
                        replica_groups=[list(range(NCORES))],
                        ins=[part[h][:, :]], outs=[rs[h][:, :]])
                if m == NM - 2:
                    # prefetch the RS_A result in the inter-collective gap
                    for mo in range(SHH // 128):
                        rta = rtp.tile([128, H], BF16, tag='rt', name=f'rta{mo}')
                        nc.sync.dma_start(
                            out=rta[:, :], in_=rs[0][mo * 128:(mo + 1) * 128, :])
                        rt_a.append(rta)

            # ---- 4. deferred shared-down runs during RS_B (all-resident)
            if DCH2 is not None:
                for k in range(KDEF, KI):
                    shared_down_k(k, DCH2[:, k - KDEF, :])

            # ---- 5. out = rs + shared_y (A-half adds overlap RS_B)
            for m in range(SH // 128):
                h, mo = (0, m) if m < SHH // 128 else (1, m - SHH // 128)
                if h == 0:
                    rt = rt_a[mo]
                else:
                    rt = rtp.tile([128, H], BF16, tag='rt', name=f'rtb{mo}')
                    nc.sync.dma_start(out=rt[:, :], in_=rs[1][mo * 128:(mo + 1) * 128, :])
                nc.vector.tensor_tensor(
                    out=YS[:, m, :], in0=YS[:, m, :], in1=rt[:, :],
                    op=mybir.AluOpType.add)
                nc.sync.dma_start(out=out[m * 128:(m + 1) * 128, :], in_=YS[:, m, :])

    _split_sync_waits(nc)
    return nc


def _dispatch(x2, router_w, routing_bias):
    """Host router + dispatch. Returns per-core (expert_id, tokensA, tokensB)
    with tokens as (token_id, weight), split at token HALF."""
    logits = x2 @ router_w + routing_bias            # [T, 7] fp32
    order = np.argsort(-logits, axis=1, kind='stable')[:, :TOP_K]
    probs = 1.0 / (1.0 + np.exp(-logits))
    rows = np.arange(T)
    s = probs[rows[:, None], order]                  # [T, 2]
    w = s / s.sum(axis=1, keepdims=True)             # renormalized combine weights

    lists = [[] for _ in range(E_ROUTED)]
    for k in range(TOP_K):
        for t, e, wt in zip(rows, order[:, k], w[:, k]):
            lists[e].append((int(t), float(wt)))

    loads = np.array([len(l) for l in lists])
    emax = int(np.argmax(loads))
    half = len(lists[emax]) // 2
    specs = []
    for c in range(E_ROUTED):
        toks = lists[c][:half] if c == emax else lists[c]
        specs.append((c, toks))
    specs.append((emax, lists[emax][half:]))
    # split each core's tokens at the token-id HALF boundary
    out = []
    for e, toks in specs:
        a = [tw for tw in toks if tw[0] < HALF]
        b = [tw for tw in toks if tw[0] >= HALF]
        out.append((e, a, b))
    return out


def _shuffle_gateup(wmat):
    """[H, I] -> [KI, 128(H-part), KH, 128(I-cols)] bf16, so the per-I-chunk
    stream DMA reads 2KB contiguous per partition."""
    return np.ascontiguousarray(
        wmat.reshape(KH, 128, KI, 128).transpose(2, 1, 0, 3).astype(ml_dtypes.bfloat16))


def _rup(n):
    return max(128, ((n + 127) // 128) * 128)


def kernel(x, router_w, routing_bias, shared_gate, shared_up, shared_down,
           routed_gate, routed_up, routed_down):
    global LAST_RESULT
    x = np.asarray(x, np.float32)
    router_w = np.asarray(router_w, np.float32)
    routing_bias = np.asarray(routing_bias, np.float32)
    x2 = x.reshape(T, H)

    specs = _dispatch(x2, router_w, routing_bias)
    capA = _rup(max(len(a) for _, a, _ in specs))
    capB = _rup(max(len(b) for _, _, b in specs))
    C = capA + capB

    bf = ml_dtypes.bfloat16
    routed_gate = np.asarray(routed_gate, np.float32)
    routed_up = np.asarray(routed_up, np.float32)
    routed_down = np.asarray(routed_down, np.float32)
    gw_s = [_shuffle_gateup(routed_gate[e]) for e in range(E_ROUTED)]
    uw_s = [_shuffle_gateup(routed_up[e]) for e in range(E_ROUTED)]
    dw_b = [np.ascontiguousarray(routed_down[e].astype(bf)) for e in range(E_ROUTED)]
    sgw_s = _shuffle_gateup(np.asarray(shared_gate, np.float32))
    suw_s = _shuffle_gateup(np.asarray(shared_up, np.float32))
    sdw_b = np.ascontiguousarray(np.asarray(shared_down, np.float32).astype(bf))

    in_maps = []
    shared_sets = []
    for c in range(NCORES):
        e, ta, tb = specs[c]
        idx_h = np.full((C,), PAD_IDX, np.int32)
        wv_h = np.zeros((C,), np.float32)
        xg = np.zeros((C, H), np.float32)
        for off, cap, toks, rebase in ((0, capA, ta, 0), (capA, capB, tb, HALF)):
            n = len(toks)
            if n:
                tok_ids = np.array([t for t, _ in toks], np.int64)
                idx_h[off:off + n] = tok_ids - rebase
                wv_h[off:off + n] = np.array([wt for _, wt in toks], np.float64)
                xg[off:off + n] = x2[tok_ids]
        # shared tokens = the rows this core will own after the two RS ops
        sset = np.concatenate([
            np.arange(c * SHH, (c + 1) * SHH),
            HALF + np.arange(c * SHH, (c + 1) * SHH)])
        shared_sets.append(sset)
        xt_all = np.concatenate([x2[sset].T, xg.T], axis=1).astype(bf)
        in_maps.append({
            'xt': np.ascontiguousarray(xt_all),
            'idx': idx_h,
            'wv': wv_h,
            'gw': gw_s[e], 'uw': uw_s[e], 'dw': dw_b[e],
            'sgw': sgw_s, 'suw': suw_s, 'sdw': sdw_b,
        })

    key = (capA, capB)
    nc = _PROG_CACHE.get(key)
    if nc is None:
        last_err = None
        for level in range(4):
            try:
                nc = _build_program(key, level)
                break
            except ValueError as e:
                last_err = e
        else:
            raise last_err
        _PROG_CACHE[key] = nc

    res = run_bass_kernel_spmd(nc, in_maps, list(range(NCORES)))
    LAST_RESULT = res

    out_full = np.empty((T, H), np.float32)
    for c in range(NCORES):
        out_full[shared_sets[c]] = res.results[c]['out']
    return out_full.reshape(B, S, H)



# revision 2
# speedup vs baseline: 1.6795x; 1.6795x over previous
"""DeepSeekMoE (B=2,S=2048,H=1024,I=2816, 7 routed experts top-2 + 1 shared) on 8 trn2 NeuronCores.

Strategy: collective-free unified expert-parallel.
  The shared expert has the same architecture as the routed experts, so every
  unit of work is "one MLP applied to one column" — a column is either a
  (token, routed-expert) slot or a (token, shared) slot.  12288 slot-columns
  total are packed into 8 cores x 2 uniform segments:
    seg1 (cap c1 = max expert load): core c < 7 carries routed expert c's
      entire token list; the remaining seg1 slots are shared-token filler.
    seg2 (cap c2): shared-token filler on every core.
  The top-2 combine weight is folded into the up-projection input on the host
  (xw = w * x), so slot outputs need no on-chip scaling, no scatter and no
  ReduceScatter: each core returns yt = down(silu(xg@G) * (xw@U)) [H, C] and
  the host sums each token's 3 slots (shared + 2 routed) — O(T*H) adds,
  ~0.01% of the FLOPs, same spirit as the host router.

  Device schedule per core (all matmuls bf16, f32 psum):
    1. gate/up: 22 I-chunks; per chunk stream 4 weight tiles (g/u x 2 segs)
       and run K=8-deep matmul groups over ~512-col tiles; silu on the scalar
       engine; h = silu(g)*u written to HT (bf16) by the vector engine.
    2. down: 8 H-chunks; per chunk stream 2 down-weight tiles (2 segs),
       K=22-deep matmul groups over the same column tiles, psum copied out on
       the scalar engine and DMAed to yt [H, C] f32.
  No collectives -> DMA streams freely; weights stream (2 expert sets/core,
  ~35 MB) far below the ~120 GB/s needed to keep pace with the PE.
"""

import math
import os
import sys
import types

import numpy as np
import ml_dtypes

for _p in ('/opt/trn_rl_repo', '/root/.axon_site/_ro/trn_rl_repo'):
    if os.path.isdir(_p) and _p not in sys.path:
        sys.path.append(_p)


def _install_profile_glue():
    """Optional: register the NTFF profile hook so trace=True/BASS_TRACE works
    under axon (the image's antenv lacks axon_hooks). Harmless if unavailable."""
    try:
        import antenv
        if 'antenv.axon_hooks' in sys.modules:
            return
        mod = types.ModuleType('antenv.axon_hooks')
        holder = [None]
        mod.set_axon_ntff_profile_hook = lambda h: holder.__setitem__(0, h)
        mod.get_axon_ntff_profile_hook = lambda: holder[0]
        sys.modules['antenv.axon_hooks'] = mod
        antenv.axon_hooks = mod
        so = '/opt/axon/libaxon_pjrt.so'
        if os.path.exists(so):
            from trn_agent_boot.trn_boot import _ntff_profile_via_ctypes
            hook = _ntff_profile_via_ctypes(so)
            if hook is not None:
                mod.set_axon_ntff_profile_hook(hook)
    except Exception:
        pass


_install_profile_glue()

import concourse.bass as bass
import concourse.mybir as mybir
from concourse.bass_utils import run_bass_kernel_spmd
from concourse.tile import TileContext

B, S, H, I = 2, 2048, 1024, 2816
E_ROUTED = 7
TOP_K = 2
T = B * S                  # 4096 tokens
NCORES = 8
KH = H // 128              # 8 contraction chunks over H (gate/up) = output chunks (down)
KI = I // 128              # 22 contraction chunks over I (down) = output chunks (gate/up)

F32 = mybir.dt.float32
BF16 = mybir.dt.bfloat16

LAST_RESULT = None         # BassKernelResults of the most recent run (for tests)

_PROG_CACHE = {}


def _split_sync_waits(nc, max_waits=1):
    """This container's walrus rejects >1 sync wait per instruction; spill
    extra waits onto same-engine NoOps placed just before the instruction."""
    for f in nc.m.functions:
        for bb in f.blocks:
            new_list = []
            changed = False
            for inst in bb.instructions:
                si = inst.sync_info
                if si is not None and si.on_wait is not None and len(si.on_wait) > max_waits:
                    waits = list(si.on_wait)
                    while len(waits) > max_waits:
                        chunk, waits = waits[:max_waits], waits[max_waits:]
                        nop = mybir.InstNoOp(
                            name=nc.get_next_instruction_name(),
                            engine=inst.engine, bass_nofuse=True,
                            sync_info=mybir.SyncInfo(on_wait=chunk, on_update=[]),
                        )
                        new_list.append(nop)
                    inst.sync_info = mybir.SyncInfo(
                        on_wait=waits, on_update=list(si.on_update or []))
                    changed = True
                new_list.append(inst)
            if changed:
                bb.instructions[:] = new_list


def _even_tiles(offset, total, width=512):
    """Split [offset, offset+total) into near-even tiles of <= width cols."""
    if total <= 0:
        return []
    n = (total + width - 1) // width
    base, rem = divmod(total, n)
    out = []
    c = offset
    for j in range(n):
        tn = base + (1 if j < rem else 0)
        out.append((c, tn))
        c += tn
    return out


def _build_program(caps):
    """Uniform SPMD program for segment capacities (c1, c2)."""
    c1, c2 = caps
    C = c1 + c2
    tiles1 = _even_tiles(0, c1)
    tiles2 = _even_tiles(c1, c2)

    nc = bass.Bass()
    xg = nc.declare_dram_parameter('xg', [H, C], BF16, isOutput=False)
    xw = nc.declare_dram_parameter('xw', [H, C], BF16, isOutput=False)
    # gate/up weights arrive chunk-shuffled: [KI, 128, KH, 128] so each
    # per-I-chunk stream DMA reads 2KB-contiguous per partition.
    g1 = nc.declare_dram_parameter('g1', [KI, 128, KH, 128], BF16, isOutput=False)
    u1 = nc.declare_dram_parameter('u1', [KI, 128, KH, 128], BF16, isOutput=False)
    g2 = nc.declare_dram_parameter('g2', [KI, 128, KH, 128], BF16, isOutput=False)
    u2 = nc.declare_dram_parameter('u2', [KI, 128, KH, 128], BF16, isOutput=False)
    # down weights shuffled per output H-chunk: [KH, 128, KI, 128]
    d1 = nc.declare_dram_parameter('d1', [KH, 128, KI, 128], BF16, isOutput=False)
    d2 = nc.declare_dram_parameter('d2', [KH, 128, KI, 128], BF16, isOutput=False)
    yt = nc.declare_dram_parameter('yt', [H, C], F32, isOutput=True)

    xg_r = xg.rearrange('(k p) c -> p k c', p=128)
    xw_r = xw.rearrange('(k p) c -> p k c', p=128)

    with TileContext(nc) as tc:
        with (
            tc.tile_pool(name='big', bufs=1) as bigp,
            tc.tile_pool(name='wstream', bufs=2) as wsp,
            tc.tile_pool(name='dstream', bufs=2) as dsp,
            tc.tile_pool(name='rtp', bufs=3) as rtp,
            tc.tile_pool(name='stg', bufs=3) as stg,
            tc.tile_pool(name='ps', bufs=8, space='PSUM') as psp,
        ):
            XG = bigp.tile([128, KH, C], BF16, tag='XG')
            XW = bigp.tile([128, KH, C], BF16, tag='XW')
            HT = bigp.tile([128, KI, C], BF16, tag='HT')

            # ---- 1. gate/up over 22 I-chunks
            for i in range(KI):
                gch1 = wsp.tile([128, KH, 128], BF16, tag='g1', name=f'g1_{i}')
                nc.sync.dma_start(out=gch1[:, :, :], in_=g1[i, :, :, :])
                gch2 = wsp.tile([128, KH, 128], BF16, tag='g2', name=f'g2_{i}')
                nc.sync.dma_start(out=gch2[:, :, :], in_=g2[i, :, :, :])
                if i == 0:
                    for k in range(KH):
                        nc.sync.dma_start(out=XG[:, k, :], in_=xg_r[:, k, :])
                uch1 = wsp.tile([128, KH, 128], BF16, tag='u1', name=f'u1_{i}')
                nc.sync.dma_start(out=uch1[:, :, :], in_=u1[i, :, :, :])
                uch2 = wsp.tile([128, KH, 128], BF16, tag='u2', name=f'u2_{i}')
                nc.sync.dma_start(out=uch2[:, :, :], in_=u2[i, :, :, :])
                if i == 0:
                    for k in range(KH):
                        nc.sync.dma_start(out=XW[:, k, :], in_=xw_r[:, k, :])
                for (gch, uch, tiles) in ((gch1, uch1, tiles1), (gch2, uch2, tiles2)):
                    for (t0, tn) in tiles:
                        gps = psp.tile([128, 512], F32, tag='ps', name=f'g{i}_{t0}')
                        for k in range(KH):
                            nc.tensor.matmul(
                                gps[:, :tn], lhsT=gch[:, k, :],
                                rhs=XG[:, k, t0:t0 + tn],
                                start=(k == 0), stop=(k == KH - 1))
                        at = rtp.tile([128, 512], F32, tag='at', name=f'at{i}_{t0}')
                        nc.scalar.activation(
                            out=at[:, :tn], in_=gps[:, :tn],
                            func=mybir.ActivationFunctionType.Silu)
                        ups = psp.tile([128, 512], F32, tag='ps', name=f'u{i}_{t0}')
                        for k in range(KH):
                            nc.tensor.matmul(
                                ups[:, :tn], lhsT=uch[:, k, :],
                                rhs=XW[:, k, t0:t0 + tn],
                                start=(k == 0), stop=(k == KH - 1))
                        nc.vector.tensor_tensor(
                            out=HT[:, i, t0:t0 + tn],
                            in0=at[:, :tn], in1=ups[:, :tn],
                            op=mybir.AluOpType.mult)

            # ---- 2. down over 8 H-chunks
            for h in range(KH):
                dch1 = dsp.tile([128, KI, 128], BF16, tag='d1', name=f'd1_{h}')
                nc.sync.dma_start(out=dch1[:, :, :], in_=d1[h, :, :, :])
                dch2 = dsp.tile([128, KI, 128], BF16, tag='d2', name=f'd2_{h}')
                nc.sync.dma_start(out=dch2[:, :, :], in_=d2[h, :, :, :])
                for (dch, tiles) in ((dch1, tiles1), (dch2, tiles2)):
                    for (t0, tn) in tiles:
                        yps = psp.tile([128, 512], F32, tag='ps', name=f'y{h}_{t0}')
                        for k in range(KI):
                            nc.tensor.matmul(
                                yps[:, :tn], lhsT=dch[:, k, :],
                                rhs=HT[:, k, t0:t0 + tn],
                                start=(k == 0), stop=(k == KI - 1))
                        yst = stg.tile([128, 512], F32, tag='yst', name=f'ys{h}_{t0}')
                        nc.scalar.copy(out=yst[:, :tn], in_=yps[:, :tn])
                        nc.sync.dma_start(
                            out=yt[h * 128:(h + 1) * 128, t0:t0 + tn],
                            in_=yst[:, :tn])

    _split_sync_waits(nc)
    return nc


def _dispatch(x2, router_w, routing_bias):
    """Host router. Returns per-expert token lists [(token, weight)...]."""
    logits = x2 @ router_w + routing_bias            # [T, 7] fp32
    order = np.argsort(-logits, axis=1, kind='stable')[:, :TOP_K]
    probs = 1.0 / (1.0 + np.exp(-logits))
    rows = np.arange(T)
    s = probs[rows[:, None], order]                  # [T, 2]
    w = s / s.sum(axis=1, keepdims=True)             # renormalized combine weights

    lists = [[] for _ in range(E_ROUTED)]
    for k in range(TOP_K):
        for t, e, wt in zip(rows, order[:, k], w[:, k]):
            lists[e].append((int(t), float(wt)))
    return lists


def _shuffle_gateup(wmat):
    """[H, I] -> [KI, 128(H-part), KH, 128(I-cols)] bf16."""
    return np.ascontiguousarray(
        wmat.reshape(KH, 128, KI, 128).transpose(2, 1, 0, 3).astype(ml_dtypes.bfloat16))


def _shuffle_down(wmat):
    """[I, H] -> [KH(h), 128(I-part), KI(k), 128(H-cols)] bf16."""
    return np.ascontiguousarray(
        wmat.reshape(KI, 128, KH, 128).transpose(2, 1, 0, 3).astype(ml_dtypes.bfloat16))


def kernel(x, router_w, routing_bias, shared_gate, shared_up, shared_down,
           routed_gate, routed_up, routed_down):
    global LAST_RESULT
    x = np.asarray(x, np.float32)
    x2 = x.reshape(T, H)

    lists = _dispatch(x2, np.asarray(router_w, np.float32),
                      np.asarray(routing_bias, np.float32))

    # pieces: split any oversized expert so every piece fits one seg1 slot
    pieces = []                       # (expert_id, [(token, weight)...])
    for e in range(E_ROUTED):
        le = lists[e]
        nsplit = max(1, (len(le) + 2047) // 2048)
        step = (len(le) + nsplit - 1) // nsplit
        for a in range(0, len(le), step):
            pieces.append((e, le[a:a + step]))
    assert len(pieces) <= NCORES, 'expert pieces exceed core count'
    c1 = max(128, max(len(toks) for _, toks in pieces))
    n_spare = NCORES - len(pieces)
    c2 = max(0, -(-(T - n_spare * c1) // NCORES))
    c2 = max(c2, 1)
    C = c1 + c2

    bf = ml_dtypes.bfloat16
    routed_gate = np.asarray(routed_gate, np.float32)
    routed_up = np.asarray(routed_up, np.float32)
    routed_down = np.asarray(routed_down, np.float32)
    gw_s = [_shuffle_gateup(routed_gate[e]) for e in range(E_ROUTED)]
    uw_s = [_shuffle_gateup(routed_up[e]) for e in range(E_ROUTED)]
    dw_s = [_shuffle_down(routed_down[e]) for e in range(E_ROUTED)]
    sg_s = _shuffle_gateup(np.asarray(shared_gate, np.float32))
    su_s = _shuffle_gateup(np.asarray(shared_up, np.float32))
    sd_s = _shuffle_down(np.asarray(shared_down, np.float32))

    # shared-token filler: spare seg1 slots first, then every core's seg2
    shared_ptr = [0]

    def take_shared(n):
        a = shared_ptr[0]
        b = min(T, a + n)
        shared_ptr[0] = b
        return np.arange(a, b)

    in_maps = []
    slot_tok = np.full((NCORES, C), -1, np.int64)
    for c in range(NCORES):
        xgf = np.zeros((C, H), np.float32)
        xwf = np.zeros((C, H), np.float32)
        if c < len(pieces):
            e, toks = pieces[c]
            n = len(toks)
            tok_ids = np.array([t for t, _ in toks], np.int64)
            wts = np.array([wt for _, wt in toks], np.float32)
            xgf[:n] = x2[tok_ids]
            xwf[:n] = x2[tok_ids] * wts[:, None]
            slot_tok[c, :n] = tok_ids
            w1g, w1u, w1d = gw_s[e], uw_s[e], dw_s[e]
        else:
            tok_ids = take_shared(c1)
            n = len(tok_ids)
            xgf[:n] = x2[tok_ids]
            xwf[:n] = x2[tok_ids]
            slot_tok[c, :n] = tok_ids
            w1g, w1u, w1d = sg_s, su_s, sd_s
        tok2 = take_shared(c2)
        n2 = len(tok2)
        xgf[c1:c1 + n2] = x2[tok2]
        xwf[c1:c1 + n2] = x2[tok2]
        slot_tok[c, c1:c1 + n2] = tok2
        in_maps.append({
            'xg': np.ascontiguousarray(xgf.T.astype(bf)),
            'xw': np.ascontiguousarray(xwf.T.astype(bf)),
            'g1': w1g, 'u1': w1u, 'd1': w1d,
            'g2': sg_s, 'u2': su_s, 'd2': sd_s,
        })
    assert shared_ptr[0] >= T, 'shared filler did not cover all tokens'

    key = (c1, c2)
    nc = _PROG_CACHE.get(key)
    if nc is None:
        nc = _build_program(key)
        _PROG_CACHE[key] = nc

    res = run_bass_kernel_spmd(nc, in_maps, list(range(NCORES)))
    LAST_RESULT = res

    # host combine: each token's 3 slots (1 shared + 2 routed) summed
    yt_flat = np.concatenate(
        [np.asarray(res.results[c]['yt'], np.float32).T for c in range(NCORES)],
        axis=0)                                            # [8*C, H]
    flat_tok = slot_tok.reshape(-1)
    valid = np.flatnonzero(flat_tok >= 0)
    order = valid[np.argsort(flat_tok[valid], kind='stable')]
    idx_mat = order.reshape(T, TOP_K + 1)                  # 3 slots per token
    out2 = yt_flat[idx_mat[:, 0]] + yt_flat[idx_mat[:, 1]] + yt_flat[idx_mat[:, 2]]
    return out2.reshape(B, S, H).astype(np.float32)
